# revision 16
# baseline (speedup 1.0000x reference)
"""Trainium2 Bass kernel for nn_CircularConvolution_5403068858821.

The reference computes result[:, :, n] += 1 for m in range(M) -> a constant
tensor of shape [N, C, L_x + M - 1] filled with M (=16.0). The inputs are
never used arithmetically, so the optimal kernel is a pure HBM fill:
each of the 8 cores memsets an SBUF tile to 16.0 once and DMA-broadcasts
it over its shard of the output. No input bytes ever touch the device.

Sharding: data-parallel over batch N=32 -> 4 batches/core; per-core output
is [4*512, 4111] = [2048, 4111] f32 (~33.7 MB of HBM writes per core).
"""

import os
import time

import numpy as np

import concourse.bass as bass
import concourse.mybir as mybir
from concourse.bass_utils import run_bass_kernel_spmd

# Problem constants (hardcoded per the grading contract).
N, C, L_X = 32, 512, 4096
M = 16
L = L_X + M - 1  # 4111
N_CORES = 8
N_SHARD = N // N_CORES  # 4 batches per core
ROWS = N_SHARD * C  # 2048 rows per core
FILL = float(M)

_CACHED_NC = None
LAST_RESULTS = None  # test harness introspection: last BassKernelResults


def _build_nc():
    """Emit the per-core Bass program: fill 2048*4111 f32 elements with 16.0.

    The shard is declared as one [128, 65776] DRAM tensor (the linear
    buffer reshapes to (4, 512, 4111) on the host; every element is the
    same constant so element order is irrelevant). A [128, 512] SBUF
    tile is memset once (~0.6us), then one dma_start with a stride-0
    (broadcast) source AP replicates it 128x across the free dim (2KB
    64B-aligned descriptors), plus one 240-column remainder DMA --
    ~33.7 MB of pure HBM writes, zero HBM reads, a single HWDGE queue.

    Measured (repeat-K wall-clock slope, all 8 cores active): ~280-420
    GB/s/core HBM write bandwidth depending on terminal load. A/B-tested
    and rejected: sync/scalar/gpsimd multi-queue splits (~1.3-1.5x
    slower, contiguous or interleaved), partition-inner dst layout
    (~1.25x slower), non-64B-aligned descriptor strides (w0=4111 ~2x
    slower); w0 512/2048/8192 tie on DMA rate, 512 minimizes the serial
    memset; gpsimd memset starts ~0.2us earlier than DVE (Pool engine is
    warm from the preamble const memsets). Cost model (TimelineSim):
    97.7 us single-pass per core.
    """
    nc = bass.Bass()
    P = 128
    cols = (ROWS // P) * L  # 65776 f32 per partition row
    W0 = 512  # memset width; bulk DMA replicates it via a stride-0 AP
    reps = cols // W0  # 128
    rem = cols - reps * W0  # 240 columns, second small DMA
    out = nc.dram_tensor("out", [P, cols], mybir.dt.float32, kind="ExternalOutput")

    with (
        nc.Block() as block,
        nc.semaphore("vsem") as vsem,
        nc.semaphore("dma_sem") as dma_sem,
        nc.sbuf_tensor("src", [P, W0], mybir.dt.float32) as src_t,
    ):
        src = (
            src_t[:]
            .rearrange("p (a w) -> p a w", a=1)
            .broadcast_to([P, reps, W0])
        )
        dst = out[:, : reps * W0].rearrange("p (r w) -> p r w", r=reps)

        @block.gpsimd
        def _(g):
            g.memset(src_t[:], FILL).then_inc(vsem, 1)

        @block.sync
        def _(s):
            s.wait_ge(vsem, 1)
            s.dma_start(out=dst, in_=src).then_inc(dma_sem, 16)
            s.dma_start(out=out[:, reps * W0 :], in_=src_t[:, :rem]).then_inc(
                dma_sem, 16
            )
            s.wait_ge(dma_sem, 32)

    return nc


def kernel(x: np.ndarray, complex_weight: np.ndarray) -> np.ndarray:
    global _CACHED_NC, LAST_RESULTS
    if _CACHED_NC is None:
        _CACHED_NC = _build_nc()

    core_ids = list(range(N_CORES))
    in_maps = [{} for _ in core_ids]

    last_err = None
    for attempt in range(3):
        if attempt:
            time.sleep(60)  # axon terminal outages observed to self-recover
        try:
            res = run_bass_kernel_spmd(_CACHED_NC, in_maps, core_ids)
        except ModuleNotFoundError:
            # BASS_TRACE set but the axon NTFF profile hook isn't installed
            # in this container; retry with tracing hard-disabled.
            os.environ["BASS_NEVER_TRACE"] = "1"
            res = run_bass_kernel_spmd(_CACHED_NC, in_maps, core_ids)
        except Exception as e:  # transient tunnel/device failure
            last_err = e
            continue
        sample = [res.results[c]["out"][::37, ::1013] for c in core_ids]
        if all((s == FILL).all() for s in sample):
            break
        last_err = RuntimeError("device output failed sampled self-check")
    else:
        raise last_err
    LAST_RESULTS = res

    shards = [res.results[c]["out"].reshape(N_SHARD, C, L) for c in core_ids]
    out = np.concatenate(shards, axis=0)
    return np.ascontiguousarray(out, dtype=np.float32)
